# revision 14
# baseline (speedup 1.0000x reference)
"""Trainium2 Bass kernel for nn_EnterpriseNeuralMemory (scatter_memory).

Sharding: data-parallel over batch — 8 batch elements, one per NeuronCore.
No collectives needed (router mean is per-batch-element and chunk pooling is
chunk-local).

Per-core algorithm (batch element b, all layouts transposed = [feature, pos]):
  logitsT = attn_w.T @ x.T          (PE, bf16, 16 pos-tiles of 512)
  E^T = exp(logitsT)                (ACT, PSUM->SBUF bf16)
  P^T = x^T * E^T                   (DVE bf16 2x)
  Z,N,M = segsum64(E^T, P^T, x^T)   (DVE binary trees, bf16->f32; E&P fused
                                     in one tile so each tree level is 1 op)
  conv_pool  = W0@(m+u/64) + W1@m + W2@(m+v/64) + conv_b
               (boundary algebra: u/v from strided firsts/lasts columns)
  router: mean of chunk-first tokens -> 2-layer MLP -> softmax(3)
  out = r0*m + r1*(N/Z) + r2*conv_pool     with m = M/64

x is streamed ONCE (transposed bf16); the chunk mean m comes out of the same
DVE tree as the softmax sums, so the PE only does the logits matmul plus the
tiny conv/router epilogue.
"""

import numpy as np
import ml_dtypes

BF16 = ml_dtypes.bfloat16

B, S, D = 8, 8192, 512
C = 64                      # chunk size
NCH = S // C                # 128 chunks
P = 128                     # partitions
DT = D // P                 # 4 feature tiles
JT = 512                    # positions per matmul tile
NJ = S // JT                # 16 pos-tiles
JC = JT // C                # 8 chunks per pos-tile
HID, NEXP = 128, 3

N_CORES = 8

_CACHE = {}


def _make_pools(ctx, tc):
    return {
        "consts": ctx.enter_context(tc.tile_pool(name="consts", bufs=1)),
        "xtp": ctx.enter_context(tc.tile_pool(name="xtp", bufs=NJ)),
        "epp": ctx.enter_context(tc.tile_pool(name="epp", bufs=2)),
        "grids": ctx.enter_context(tc.tile_pool(name="grids", bufs=1)),
        "scratch": ctx.enter_context(tc.tile_pool(name="scratch", bufs=1)),
        "ps_lg": ctx.enter_context(tc.tile_pool(name="ps_lg", bufs=6, space="PSUM")),
        "ps_epi": ctx.enter_context(tc.tile_pool(name="ps_epi", bufs=2, space="PSUM")),
    }


def _emit_body(pools, nc, tc, dram, mybir):
    """Emit one full forward pass for one core."""
    f32 = mybir.dt.float32
    bf16 = mybir.dt.bfloat16
    AF = mybir.ActivationFunctionType
    OP = mybir.AluOpType
    AX = mybir.AxisListType

    consts = pools["consts"]
    xtp = pools["xtp"]
    epp = pools["epp"]
    grids = pools["grids"]
    scratch = pools["scratch"]
    ps_lg = pools["ps_lg"]
    ps_epi = pools["ps_epi"]

    # [512, X] dram tensors load as one [128, 4, X] tile each (one DMA).
    def load4(src, cols, dtype, nm):
        t = consts.tile([P, DT, cols], dtype, tag=nm, name=nm)
        nc.sync.dma_start(
            out=t[:], in_=src[:, :].rearrange("(a p) c -> p a c", p=P))
        return t

    # first stream tile + attention weights up front so PE starts ASAP
    xt0 = xtp.tile([P, DT, JT], bf16, tag="xt", name="xt0")
    nc.sync.dma_start(
        out=xt0[:],
        in_=dram["xT"][:, 0:JT].rearrange("(a p) c -> p a c", p=P))

    aw = []
    for k in range(DT):
        t = consts.tile([P, D], bf16, tag=f"aw{k}", name=f"aw{k}")
        nc.sync.dma_start(out=t[:], in_=dram["attn_w"][k * P:(k + 1) * P, :])
        aw.append(t)

    # router / boundary inputs (small; needed within the first few tiles)
    fp4 = load4(dram["fpad"], NCH + 1, f32, "fp4")
    lp4 = load4(dram["lpad"], NCH + 1, f32, "lp4")
    rw14 = load4(dram["router_w1"], HID, f32, "rw14")
    rw1 = [rw14[:, k] for k in range(DT)]
    rb1 = consts.tile([1, HID], f32, tag="rb1", name="rb1")
    nc.sync.dma_start(out=rb1[:], in_=dram["router_b1"][:])
    rw2 = consts.tile([HID, NEXP], f32, tag="rw2", name="rw2")
    nc.sync.dma_start(out=rw2[:], in_=dram["router_w2"][:])
    rb2 = consts.tile([1, NEXP], f32, tag="rb2", name="rb2")
    nc.sync.dma_start(out=rb2[:], in_=dram["router_b2"][:])
    ones11 = consts.tile([1, 1], f32, tag="ones11", name="ones11")
    nc.vector.memset(ones11[:], 1.0)
    ones1p = consts.tile([1, P], f32, tag="ones1p", name="ones1p")
    nc.vector.memset(ones1p[:], 1.0)

    # conv boundary terms: u_i = L_{i-1}-L_i, v_i = F_{i+1}-F_i
    u = grids.tile([P, DT, NCH], f32, tag="u", name="u")
    nc.vector.tensor_tensor(out=u[:], in0=lp4[:, :, 0:NCH],
                            in1=lp4[:, :, 1:NCH + 1], op=OP.subtract)
    v = grids.tile([P, DT, NCH], f32, tag="v", name="v")
    nc.vector.tensor_tensor(out=v[:], in0=fp4[:, :, 1:NCH + 1],
                            in1=fp4[:, :, 0:NCH], op=OP.subtract)

    def emit_router():
        # router MLP + softmax + broadcast of r; emitted after tile 0's
        # matmuls so its PE ops never block the stream start
        xfs = grids.tile([P, DT], f32, tag="xfs", name="xfs")
        nc.vector.reduce_sum(out=xfs[:], in_=fp4[:, :, 0:NCH], axis=AX.X)
        xf = grids.tile([P, DT], f32, tag="xf", name="xf")
        nc.scalar.mul(xf[:], xfs[:], 1.0 / NCH)
        ps_h = ps_epi.tile([P, 1], f32, tag="epi", name="epi")
        for k in range(DT):
            nc.tensor.matmul(ps_h[:], rw1[k][:], xf[:, k:k + 1],
                             start=(k == 0), stop=False)
        nc.tensor.matmul(ps_h[:], rb1[:], ones11[:], start=False, stop=True)
        hsb = grids.tile([P, 1], f32, tag="hsb", name="hsb")
        nc.scalar.activation(out=hsb[:], in_=ps_h[:], func=AF.Relu)
        ps_r = ps_epi.tile([1, NEXP], f32, tag="epi", name="epi")
        nc.tensor.matmul(ps_r[:], hsb[:], rw2[:], start=True, stop=False)
        nc.tensor.matmul(ps_r[:], ones11[:], rb2[:], start=False, stop=True)
        rmax = grids.tile([1, 1], f32, tag="rmax", name="rmax")
        nc.vector.reduce_max(out=rmax[:], in_=ps_r[:], axis=AX.X)
        nrmax = grids.tile([1, 1], f32, tag="nrmax", name="nrmax")
        nc.vector.tensor_scalar_mul(nrmax[:], rmax[:], -1.0)
        er = grids.tile([1, NEXP], f32, tag="er", name="er")
        nc.scalar.activation(out=er[:], in_=ps_r[:], func=AF.Exp,
                             bias=nrmax[:])
        rsum = grids.tile([1, 1], f32, tag="rsum", name="rsum")
        nc.vector.reduce_sum(out=rsum[:], in_=er[:], axis=AX.X)
        rrec = grids.tile([1, 1], f32, tag="rrec", name="rrec")
        nc.vector.reciprocal(rrec[:], rsum[:])
        rvec = grids.tile([1, NEXP], f32, tag="rvec", name="rvec")
        nc.vector.tensor_scalar_mul(rvec[:], er[:], rrec[:])
        ps_b = ps_epi.tile([P, NEXP], f32, tag="epi", name="epi")
        nc.tensor.matmul(ps_b[:], ones1p[:], rvec[:], start=True, stop=True)
        rb = grids.tile([P, NEXP], f32, tag="rb", name="rb")
        nc.scalar.copy(rb[:], ps_b[:])
        return rb

    # segsum result grids: ZN[:,0:4]=Z (softmax denom), ZN[:,4:8]=N (numer)
    ZN = grids.tile([P, 2 * DT, NCH], f32, tag="ZN", name="ZN")
    Mc = grids.tile([P, DT, NCH], f32, tag="Mc", name="Mc")

    # epilogue grids (written in chunk-range halves)
    mT = grids.tile([P, DT, NCH], f32, tag="mT", name="mT")
    mTb = grids.tile([P, DT, NCH], bf16, tag="mTb", name="mTb")
    aTb = grids.tile([P, DT, NCH], bf16, tag="aTb", name="aTb")
    cTb = grids.tile([P, DT, NCH], bf16, tag="cTb", name="cTb")
    convT = grids.tile([P, DT, NCH], f32, tag="convT", name="convT")
    rz = grids.tile([P, DT, NCH], f32, tag="rz", name="rz")
    attnT = grids.tile([P, DT, NCH], f32, tag="attnT", name="attnT")
    tmp = grids.tile([P, DT, NCH], f32, tag="tmp", name="tmp")
    acc = grids.tile([P, DT, NCH], f32, tag="acc", name="acc")
    y4 = grids.tile([P, DT, NCH], f32, tag="y4", name="y4")

    wT = {}
    cbr = consts.tile([1, D], f32, tag="cbr", name="cbr")

    def emit_wT_loads():
        # conv-expert weights: deferred so they queue behind the first
        # stream tiles instead of delaying them
        for w in range(3):
            w4 = load4(dram[f"w{w}T"], D, bf16, f"w{w}T4")
            wT[w] = [w4[:, k] for k in range(DT)]
        nc.sync.dma_start(out=cbr[:], in_=dram["conv_b_row"][:])

    def epi_prep(c0, c1):
        # conv-expert inputs for chunk range [c0, c1): m, m+u/64, m+v/64
        nc.scalar.mul(mT[:, :, c0:c1], Mc[:, :, c0:c1], 1.0 / C)
        nc.scalar.copy(mTb[:, :, c0:c1], mT[:, :, c0:c1])
        nc.vector.scalar_tensor_tensor(
            out=aTb[:, :, c0:c1], in0=u[:, :, c0:c1], scalar=1.0 / C,
            in1=mT[:, :, c0:c1], op0=OP.mult, op1=OP.add)
        nc.vector.scalar_tensor_tensor(
            out=cTb[:, :, c0:c1], in0=v[:, :, c0:c1], scalar=1.0 / C,
            in1=mT[:, :, c0:c1], op0=OP.mult, op1=OP.add)

    def epi_conv(c0, c1):
        # conv expert matmuls for chunk range [c0, c1)
        n = c1 - c0
        for o in range(DT):
            ps = ps_epi.tile([P, n], f32, tag="epi", name="epi")
            first = True
            for w, rhs4 in ((0, aTb), (1, mTb), (2, cTb)):
                for k in range(DT):
                    nc.tensor.matmul(
                        ps[:], wT[w][k][:, o * P:(o + 1) * P],
                        rhs4[:, k, c0:c1], start=first, stop=False)
                    first = False
            nc.tensor.matmul(
                ps[:], cbr[:, o * P:(o + 1) * P], ones1p[:, 0:n],
                start=False, stop=True)
            nc.scalar.copy(convT[:, o, c0:c1], ps[:])

    def epi_mix(c0, c1, rb):
        # attention division + routed mix + output DMA for [c0, c1)
        nc.vector.reciprocal(rz[:, :, c0:c1], ZN[:, 0:DT, c0:c1])
        nc.vector.tensor_tensor(out=attnT[:, :, c0:c1],
                                in0=ZN[:, DT:2 * DT, c0:c1],
                                in1=rz[:, :, c0:c1], op=OP.mult)
        nc.scalar.mul(tmp[:, :, c0:c1], mT[:, :, c0:c1], rb[:, 0:1])
        nc.vector.scalar_tensor_tensor(
            out=acc[:, :, c0:c1], in0=attnT[:, :, c0:c1], scalar=rb[:, 1:2],
            in1=tmp[:, :, c0:c1], op0=OP.mult, op1=OP.add)
        nc.vector.scalar_tensor_tensor(
            out=y4[:, :, c0:c1], in0=convT[:, :, c0:c1], scalar=rb[:, 2:3],
            in1=acc[:, :, c0:c1], op0=OP.mult, op1=OP.add)
        nc.sync.dma_start(
            out=dram["y"][:, c0:c1].rearrange("(a p) n -> p a n", p=P),
            in_=y4[:, :, c0:c1])

    # ---------------- x-stream DMAs up front ----------------
    # One SBUF buffer per tile: DMA is never gated by buffer recycling, so
    # the Pool engine's x chunk-sum chains are never starved for input.
    xts = [xt0]
    for j in range(1, NJ):
        xt = xtp.tile([P, DT, JT], bf16, tag="xt", name="xt")
        nc.sync.dma_start(
            out=xt[:],
            in_=dram["xT"][:, j * JT:(j + 1) * JT].rearrange(
                "(a p) c -> p a c", p=P))
        xts.append(xt)

    emit_wT_loads()

    t3s = {}

    def pool_chain(j):
        # x segsum64 pair-adds for tile j on the Pool engine
        xv = xts[j][:].rearrange("p a (n c) -> p a n c", c=C)
        t1 = scratch.tile([P, DT, JC, C // 2], bf16, tag="t1",
                          name="t1", bufs=2)
        nc.gpsimd.tensor_tensor(out=t1[:], in0=xv[:, :, :, 0:32],
                                in1=xv[:, :, :, 32:64], op=OP.add)
        t2 = scratch.tile([P, DT, JC, C // 4], bf16, tag="t2",
                          name="t2", bufs=2)
        nc.gpsimd.tensor_tensor(out=t2[:], in0=t1[:, :, :, 0:16],
                                in1=t1[:, :, :, 16:32], op=OP.add)
        t3 = scratch.tile([P, DT, JC, C // 8], bf16, tag="t3",
                          name="t3", bufs=NJ)
        nc.gpsimd.tensor_tensor(out=t3[:], in0=t2[:, :, :, 0:8],
                                in1=t2[:, :, :, 8:16], op=OP.add)
        t3s[j] = t3

    def mc_red(j):
        nc.vector.reduce_sum(out=Mc[:, :, j * JC:(j + 1) * JC],
                             in_=t3s[j][:], axis=AX.X)

    # ---------------- main streaming phase ----------------
    # Epilogue quarters are pipelined across the stream: quarter q's chunks
    # complete at tile 4q+3; prep/conv/mix are spread over later tiles so no
    # engine ever stalls in-order on a cross-engine epilogue dependency.
    QC = NCH // 4
    rb = None
    for j in range(NJ):
        xt = xts[j]
        pool_chain(j)

        # EP[:,0:4]=E^T (exp of logits), EP[:,4:8]=P^T (x*E)
        EP = epp.tile([P, 2 * DT, JT], bf16, tag="EP", name="EP")
        for o in range(DT):
            ps = ps_lg.tile([P, JT], f32, tag="lg", name="lg")
            for k in range(DT):
                nc.tensor.matmul(
                    ps[:], aw[k][:, o * P:(o + 1) * P], xt[:, k],
                    start=(k == 0), stop=(k == DT - 1))
            nc.scalar.activation(out=EP[:, o, :], in_=ps[:], func=AF.Exp)
            if j == 0:
                # startup: per-o mult so DVE begins right after the first exp
                nc.vector.tensor_tensor(
                    out=EP[:, DT + o, :], in0=xt[:, o], in1=EP[:, o, :],
                    op=OP.mult)
        if j > 0:
            nc.vector.tensor_tensor(
                out=EP[:, DT:2 * DT, :], in0=xt[:], in1=EP[:, 0:DT, :],
                op=OP.mult)

        # E&P segsum64 (DVE): three bf16 pair-add levels (2x mode), then a
        # small f32 reduce over the remaining 8
        ch0 = j * JC
        epv = EP[:].rearrange("p a (n c) -> p a n c", c=C)
        s1 = scratch.tile([P, 2 * DT, JC, C // 2], bf16, tag="s1",
                          name="s1", bufs=2)
        nc.vector.tensor_tensor(out=s1[:], in0=epv[:, :, :, 0:32],
                                in1=epv[:, :, :, 32:64], op=OP.add)
        s2 = scratch.tile([P, 2 * DT, JC, C // 4], bf16, tag="s2",
                          name="s2", bufs=2)
        nc.vector.tensor_tensor(out=s2[:], in0=s1[:, :, :, 0:16],
                                in1=s1[:, :, :, 16:32], op=OP.add)
        s3 = scratch.tile([P, 2 * DT, JC, C // 8], bf16, tag="s3",
                          name="s3", bufs=2)
        nc.vector.tensor_tensor(out=s3[:], in0=s2[:, :, :, 0:8],
                                in1=s2[:, :, :, 8:16], op=OP.add)
        nc.vector.reduce_sum(out=ZN[:, :, ch0:ch0 + JC], in_=s3[:], axis=AX.X)

        # deferred x chunk-sum reduces (one tile of slack behind Pool)
        if 1 <= j <= 14:
            mc_red(j - 1)
        elif j == 15:
            mc_red(14)
            mc_red(15)

        if j == 0:
            rb = emit_router()
        elif j in (4, 8, 12):
            epi_prep((j - 4) // 4 * QC, (j - 4) // 4 * QC + QC)
        elif j in (5, 9, 13):
            epi_conv((j - 5) // 4 * QC, (j - 5) // 4 * QC + QC)
        elif j in (6, 10, 14):
            epi_mix((j - 6) // 4 * QC, (j - 6) // 4 * QC + QC, rb)

    # final quarter (everything else was emitted mid-stream)
    epi_prep(3 * QC, NCH)
    epi_conv(3 * QC, NCH)
    epi_mix(3 * QC, NCH, rb)


def _build(loop_iters=None):
    import concourse.bass as bass
    from concourse import bacc
    import concourse.mybir as mybir
    import concourse.tile as tile

    f32 = mybir.dt.float32
    bf16 = mybir.dt.bfloat16

    nc = bacc.Bacc(None, target_bir_lowering=False)
    dram = {
        "xT": nc.dram_tensor("xT", [D, S], bf16, kind="ExternalInput"),
        "attn_w": nc.dram_tensor("attn_w", [D, D], bf16, kind="ExternalInput"),
        "w0T": nc.dram_tensor("w0T", [D, D], bf16, kind="ExternalInput"),
        "w1T": nc.dram_tensor("w1T", [D, D], bf16, kind="ExternalInput"),
        "w2T": nc.dram_tensor("w2T", [D, D], bf16, kind="ExternalInput"),
        "fpad": nc.dram_tensor("fpad", [D, NCH + 1], f32, kind="ExternalInput"),
        "lpad": nc.dram_tensor("lpad", [D, NCH + 1], f32, kind="ExternalInput"),
        "router_w1": nc.dram_tensor("router_w1", [D, HID], f32, kind="ExternalInput"),
        "router_b1": nc.dram_tensor("router_b1", [1, HID], f32, kind="ExternalInput"),
        "router_w2": nc.dram_tensor("router_w2", [HID, NEXP], f32, kind="ExternalInput"),
        "router_b2": nc.dram_tensor("router_b2", [1, NEXP], f32, kind="ExternalInput"),
        "conv_b_row": nc.dram_tensor("conv_b_row", [1, D], f32, kind="ExternalInput"),
        "y": nc.dram_tensor("y", [D, NCH], f32, kind="ExternalOutput"),
    }
    from contextlib import ExitStack
    with tile.TileContext(nc) as tc:
        with ExitStack() as ctx:
            pools = _make_pools(ctx, tc)
            if loop_iters is None:
                _emit_body(pools, nc, tc, dram, mybir)
            else:
                ET = mybir.EngineType
                with tc.For_i(0, loop_iters, 1,
                              hint_engines=(ET.PE, ET.DVE, ET.Activation,
                                            ET.Pool, ET.SP)):
                    _emit_body(pools, nc, tc, dram, mybir)
    nc.finalize()
    return nc


def _host_prep(inputs):
    """Build per-core input maps from full inputs."""
    x = np.asarray(inputs["x"], dtype=np.float32)
    attn_w = np.asarray(inputs["attn_w"], dtype=np.float32)
    conv_w = np.asarray(inputs["conv_w"], dtype=np.float32)
    conv_b = np.asarray(inputs["conv_b"], dtype=np.float32)
    rw1 = np.asarray(inputs["router_w1"], dtype=np.float32)
    rb1 = np.asarray(inputs["router_b1"], dtype=np.float32)
    rw2 = np.asarray(inputs["router_w2"], dtype=np.float32)
    rb2 = np.asarray(inputs["router_b2"], dtype=np.float32)

    aw_bf = np.ascontiguousarray(attn_w).astype(BF16)
    w0T = np.ascontiguousarray(conv_w[:, :, 0].T).astype(BF16)
    w1T = np.ascontiguousarray(conv_w[:, :, 1].T).astype(BF16)
    w2T = np.ascontiguousarray(conv_w[:, :, 2].T).astype(BF16)
    rb1_2d = rb1.reshape(1, HID)
    rb2_2d = rb2.reshape(1, NEXP)
    cb_row = conv_b.reshape(1, D)

    in_maps = []
    for b in range(B):
        xb = x[b]
        F = xb[0::C]            # [NCH, D]
        L = xb[C - 1::C]
        fpad = np.zeros((D, NCH + 1), np.float32)
        fpad[:, 0:NCH] = F.T
        lpad = np.zeros((D, NCH + 1), np.float32)
        lpad[:, 1:NCH + 1] = L.T
        in_maps.append({
            "xT": np.ascontiguousarray(xb.T).astype(BF16),
            "attn_w": aw_bf,
            "w0T": w0T, "w1T": w1T, "w2T": w2T,
            "fpad": fpad, "lpad": lpad,
            "router_w1": rw1, "router_b1": rb1_2d,
            "router_w2": rw2, "router_b2": rb2_2d,
            "conv_b_row": cb_row,
        })
    return in_maps


def kernel(**inputs):
    from concourse.bass_utils import run_bass_kernel_spmd

    if "nc" not in _CACHE:
        _CACHE["nc"] = _build()
    nc = _CACHE["nc"]
    in_maps = _host_prep(inputs)
    res = run_bass_kernel_spmd(nc, in_maps, list(range(N_CORES)))
    out = np.stack([np.ascontiguousarray(res.results[b]["y"].T)
                    for b in range(B)])
    return out.astype(np.float32)


if __name__ == "__main__":
    rng = np.random.default_rng(0)
    fake = {
        "x": rng.standard_normal((B, S, D), dtype=np.float32),
        "attn_w": rng.standard_normal((D, D), dtype=np.float32) / np.sqrt(D),
        "attn_b": np.zeros(D, np.float32),
        "conv_w": rng.standard_normal((D, D, 3), dtype=np.float32) / np.sqrt(3 * D),
        "conv_b": np.zeros(D, np.float32),
        "router_w1": rng.standard_normal((D, HID), dtype=np.float32) / np.sqrt(D),
        "router_b1": np.zeros(HID, np.float32),
        "router_w2": rng.standard_normal((HID, NEXP), dtype=np.float32) / np.sqrt(HID),
        "router_b2": np.zeros(NEXP, np.float32),
    }
    y = kernel(**fake)
    print("kernel out", y.shape, y.dtype, np.abs(y).max())


# revision 17
# speedup vs baseline: 1.1981x; 1.1981x over previous
"""Trainium2 Bass kernel for nn_EnterpriseNeuralMemory (scatter_memory).

Sharding: data-parallel over batch — 8 batch elements, one per NeuronCore.
No collectives needed (router mean is per-batch-element and chunk pooling is
chunk-local).

Per-core algorithm (batch element b, transposed layouts = [feature, pos]):
  logitsT = attn_w.T @ x.T          (PE, bf16, 16 pos-tiles of 512)
  E^T = exp(logitsT)                (ACT, PSUM->SBUF bf16)
  P^T = x^T * E^T                   (DVE bf16 2x)
  Z,N  = segsum64(E^T, P^T)         (DVE: 3 bf16 pair-add levels + f32 red)
  m    = chunk-sums of x on PE      (block-ones matmuls over a natural-layout
                                     xn stream, PSUM accum in 2 half-banks,
                                     then PE-transpose to [d, chunk])
  conv_pool  = W0@(m+u/64) + W1@m + W2@(m+v/64) + conv_b
               (boundary algebra: u/v from strided firsts/lasts columns)
  router: mean of chunk-first tokens -> 2-layer MLP -> softmax(3)
  out = r0*m + r1*(N/Z) + r2*conv_pool

Engine budget notes: DVE and Pool(GpSimd) share SBUF ports on real HW, so
Pool offloading is useless — everything elementwise rides the DVE port; the
x chunk-sums therefore go to PE (own port), exp/copies to ACT (own port).
"""

import numpy as np
import ml_dtypes

BF16 = ml_dtypes.bfloat16

B, S, D = 8, 8192, 512
C = 64                      # chunk size
NCH = S // C                # 128 chunks
P = 128                     # partitions
DT = D // P                 # 4 feature tiles
JT = 512                    # positions per matmul tile
NJ = S // JT                # 16 pos-tiles
JC = JT // C                # 8 chunks per pos-tile
HID, NEXP = 128, 3

N_CORES = 8

_CACHE = {}


def _make_pools(ctx, tc):
    return {
        "consts": ctx.enter_context(tc.tile_pool(name="consts", bufs=1)),
        "xtp": ctx.enter_context(tc.tile_pool(name="xtp", bufs=6)),
        "xnp": ctx.enter_context(tc.tile_pool(name="xnp", bufs=6)),
        "epp": ctx.enter_context(tc.tile_pool(name="epp", bufs=2)),
        "grids": ctx.enter_context(tc.tile_pool(name="grids", bufs=1)),
        "scratch": ctx.enter_context(tc.tile_pool(name="scratch", bufs=1)),
        "ps_lg": ctx.enter_context(tc.tile_pool(name="ps_lg", bufs=5, space="PSUM")),
        "ps_m": ctx.enter_context(tc.tile_pool(name="ps_m", bufs=2, space="PSUM")),
        "ps_epi": ctx.enter_context(tc.tile_pool(name="ps_epi", bufs=1, space="PSUM")),
    }


def _emit_body(pools, nc, tc, dram, mybir):
    """Emit one full forward pass for one core."""
    f32 = mybir.dt.float32
    bf16 = mybir.dt.bfloat16
    AF = mybir.ActivationFunctionType
    OP = mybir.AluOpType
    AX = mybir.AxisListType

    consts = pools["consts"]
    xtp = pools["xtp"]
    xnp = pools["xnp"]
    epp = pools["epp"]
    grids = pools["grids"]
    scratch = pools["scratch"]
    ps_lg = pools["ps_lg"]
    ps_m = pools["ps_m"]
    ps_epi = pools["ps_epi"]

    # [512, X] dram tensors load as one [128, 4, X] tile each (one DMA).
    def load4(src, cols, dtype, nm):
        t = consts.tile([P, DT, cols], dtype, tag=nm, name=nm)
        nc.sync.dma_start(
            out=t[:], in_=src[:, :].rearrange("(a p) c -> p a c", p=P))
        return t

    # first stream tile + attention weights up front so PE starts ASAP
    xt0 = xtp.tile([P, DT, JT], bf16, tag="xt", name="xt0")
    nc.sync.dma_start(
        out=xt0[:],
        in_=dram["xT"][:, 0:JT].rearrange("(a p) c -> p a c", p=P))

    aw = []
    for k in range(DT):
        t = consts.tile([P, D], bf16, tag=f"aw{k}", name=f"aw{k}")
        nc.sync.dma_start(out=t[:], in_=dram["attn_w"][k * P:(k + 1) * P, :])
        aw.append(t)

    G = consts.tile([P, 2 * P], bf16, tag="G", name="G")
    nc.sync.dma_start(out=G[:], in_=dram["G"][:])
    ident = consts.tile([P, P], bf16, tag="ident", name="ident")
    nc.sync.dma_start(out=ident[:], in_=dram["ident"][:])

    # router / boundary inputs (small; needed within the first few tiles)
    fp4 = load4(dram["fpad"], NCH + 1, f32, "fp4")
    lp4 = load4(dram["lpad"], NCH + 1, f32, "lp4")
    rw14 = load4(dram["router_w1"], HID, f32, "rw14")
    rw1 = [rw14[:, k] for k in range(DT)]
    rb1 = consts.tile([1, HID], f32, tag="rb1", name="rb1")
    nc.sync.dma_start(out=rb1[:], in_=dram["router_b1"][:])
    rw2 = consts.tile([HID, NEXP], f32, tag="rw2", name="rw2")
    nc.sync.dma_start(out=rw2[:], in_=dram["router_w2"][:])
    rb2 = consts.tile([1, NEXP], f32, tag="rb2", name="rb2")
    nc.sync.dma_start(out=rb2[:], in_=dram["router_b2"][:])
    ones11 = consts.tile([1, 1], f32, tag="ones11", name="ones11")
    nc.vector.memset(ones11[:], 1.0)
    ones1p = consts.tile([1, P], f32, tag="ones1p", name="ones1p")
    nc.vector.memset(ones1p[:], 1.0)

    # remaining stream DMAs: xT tiles interleaved with natural-layout tiles
    xts = [xt0]
    xns = []
    for j in range(1, NJ):
        xt = xtp.tile([P, DT, JT], bf16, tag="xt", name="xt")
        nc.sync.dma_start(
            out=xt[:],
            in_=dram["xT"][:, j * JT:(j + 1) * JT].rearrange(
                "(a p) c -> p a c", p=P))
        xts.append(xt)
        xn = xnp.tile([P, DT, JT], bf16, tag="xn", name="xn")
        nc.sync.dma_start(
            out=xn[:],
            in_=dram["xn"][(j - 1) * JT:j * JT, :].rearrange(
                "(b p) d -> p b d", p=P))
        xns.append(xn)
    xn = xnp.tile([P, DT, JT], bf16, tag="xn", name="xn")
    nc.sync.dma_start(
        out=xn[:],
        in_=dram["xn"][(NJ - 1) * JT:NJ * JT, :].rearrange(
            "(b p) d -> p b d", p=P))
    xns.append(xn)

    # conv weights (needed from the first epilogue half onward)
    wT = {}
    for w in range(3):
        w4 = load4(dram[f"w{w}T"], D, bf16, f"w{w}T4")
        wT[w] = [w4[:, k] for k in range(DT)]
    cbr = consts.tile([1, D], f32, tag="cbr", name="cbr")
    nc.sync.dma_start(out=cbr[:], in_=dram["conv_b_row"][:])

    # conv boundary terms: u_i = L_{i-1}-L_i, v_i = F_{i+1}-F_i
    u = grids.tile([P, DT, NCH], f32, tag="u", name="u")
    nc.vector.tensor_tensor(out=u[:], in0=lp4[:, :, 0:NCH],
                            in1=lp4[:, :, 1:NCH + 1], op=OP.subtract)
    v = grids.tile([P, DT, NCH], f32, tag="v", name="v")
    nc.vector.tensor_tensor(out=v[:], in0=fp4[:, :, 1:NCH + 1],
                            in1=fp4[:, :, 0:NCH], op=OP.subtract)

    def emit_router():
        # router MLP + softmax + broadcast of r; emitted after tile 0's
        # matmuls so its PE ops never block the stream start
        xfs = grids.tile([P, DT], f32, tag="xfs", name="xfs")
        nc.vector.reduce_sum(out=xfs[:], in_=fp4[:, :, 0:NCH], axis=AX.X)
        xf = grids.tile([P, DT], f32, tag="xf", name="xf")
        nc.scalar.mul(xf[:], xfs[:], 1.0 / NCH)
        ps_h = ps_epi.tile([P, 1], f32, tag="epi", name="epi")
        for k in range(DT):
            nc.tensor.matmul(ps_h[:], rw1[k][:], xf[:, k:k + 1],
                             start=(k == 0), stop=False)
        nc.tensor.matmul(ps_h[:], rb1[:], ones11[:], start=False, stop=True)
        hsb = grids.tile([P, 1], f32, tag="hsb", name="hsb")
        nc.scalar.activation(out=hsb[:], in_=ps_h[:], func=AF.Relu)
        ps_r = ps_epi.tile([1, NEXP], f32, tag="epi", name="epi")
        nc.tensor.matmul(ps_r[:], hsb[:], rw2[:], start=True, stop=False)
        nc.tensor.matmul(ps_r[:], ones11[:], rb2[:], start=False, stop=True)
        rmax = grids.tile([1, 1], f32, tag="rmax", name="rmax")
        nc.vector.reduce_max(out=rmax[:], in_=ps_r[:], axis=AX.X)
        nrmax = grids.tile([1, 1], f32, tag="nrmax", name="nrmax")
        nc.vector.tensor_scalar_mul(nrmax[:], rmax[:], -1.0)
        er = grids.tile([1, NEXP], f32, tag="er", name="er")
        nc.scalar.activation(out=er[:], in_=ps_r[:], func=AF.Exp,
                             bias=nrmax[:])
        rsum = grids.tile([1, 1], f32, tag="rsum", name="rsum")
        nc.vector.reduce_sum(out=rsum[:], in_=er[:], axis=AX.X)
        rrec = grids.tile([1, 1], f32, tag="rrec", name="rrec")
        nc.vector.reciprocal(rrec[:], rsum[:])
        rvec = grids.tile([1, NEXP], f32, tag="rvec", name="rvec")
        nc.vector.tensor_scalar_mul(rvec[:], er[:], rrec[:])
        ps_b = ps_epi.tile([P, NEXP], f32, tag="epi", name="epi")
        nc.tensor.matmul(ps_b[:], ones1p[:], rvec[:], start=True, stop=True)
        rb = grids.tile([P, NEXP], f32, tag="rb", name="rb")
        nc.scalar.copy(rb[:], ps_b[:])
        return rb

    # segsum result grids: ZN[:,0:4]=Z (softmax denom), ZN[:,4:8]=N (numer)
    ZN = grids.tile([P, 2 * DT, NCH], f32, tag="ZN", name="ZN")

    # epilogue grids (written in chunk-range halves)
    mT = grids.tile([P, DT, NCH], f32, tag="mT", name="mT")
    mTb = grids.tile([P, DT, NCH], bf16, tag="mTb", name="mTb")
    aTb = grids.tile([P, DT, NCH], bf16, tag="aTb", name="aTb")
    cTb = grids.tile([P, DT, NCH], bf16, tag="cTb", name="cTb")
    convT = grids.tile([P, DT, NCH], f32, tag="convT", name="convT")
    rz = grids.tile([P, DT, NCH], f32, tag="rz", name="rz")
    attnT = grids.tile([P, DT, NCH], f32, tag="attnT", name="attnT")
    tmp = grids.tile([P, DT, NCH], f32, tag="tmp", name="tmp")
    acc = grids.tile([P, DT, NCH], f32, tag="acc", name="acc")
    y4 = grids.tile([P, DT, NCH], f32, tag="y4", name="y4")

    # m accumulation: two PSUM half-banks, chunks 0-63 and 64-127 at
    # partitions 0..63 (G block-ones matmuls over the natural stream)
    mbank = [ps_m.tile([P, D], f32, tag="mb", name=f"mb{h}") for h in (0, 1)]

    def gmm(j):
        # 4 block-ones matmuls: chunk sums of tile j into its half-bank
        half, lj = j // 8, j % 8
        for b in range(DT):
            i = lj * DT + b     # chunk-pair index within the half (0..31)
            nc.tensor.matmul(
                mbank[half][:], G[:, P - 2 * i:2 * P - 2 * i], xns[j][:, b],
                start=(i == 0), stop=(i == 31), skip_group_check=True)

    def m_close(half):
        # PSUM [64 chunks, 512 d] -> mT[d, chunks half] via scaled copy +
        # four PE transposes
        c0 = half * (NCH // 2)
        m_nat = scratch.tile([P, D], bf16, tag="m_nat", name="m_nat", bufs=2)
        nc.scalar.mul(m_nat[0:64, :], mbank[half][0:64, :], 1.0 / C)
        for k in range(DT):
            pst = ps_epi.tile([P, P], bf16, tag="epi", name="epi")
            nc.tensor.transpose(pst[:], m_nat[:, k * P:(k + 1) * P], ident[:])
            nc.scalar.copy(mT[:, k, c0:c0 + NCH // 2], pst[:, 0:64])
        nc.scalar.copy(mTb[:, :, c0:c0 + NCH // 2], mT[:, :, c0:c0 + NCH // 2])

    def epi_prep(c0, c1):
        # conv-expert inputs for chunk range [c0, c1): m+u/64, m+v/64
        nc.vector.scalar_tensor_tensor(
            out=aTb[:, :, c0:c1], in0=u[:, :, c0:c1], scalar=1.0 / C,
            in1=mT[:, :, c0:c1], op0=OP.mult, op1=OP.add)
        nc.vector.scalar_tensor_tensor(
            out=cTb[:, :, c0:c1], in0=v[:, :, c0:c1], scalar=1.0 / C,
            in1=mT[:, :, c0:c1], op0=OP.mult, op1=OP.add)

    def epi_conv(c0, c1):
        # conv expert matmuls for chunk range [c0, c1)
        n = c1 - c0
        for o in range(DT):
            ps = ps_epi.tile([P, n], f32, tag="epi", name="epi")
            first = True
            for w, rhs4 in ((0, aTb), (1, mTb), (2, cTb)):
                for k in range(DT):
                    nc.tensor.matmul(
                        ps[:], wT[w][k][:, o * P:(o + 1) * P],
                        rhs4[:, k, c0:c1], start=first, stop=False)
                    first = False
            nc.tensor.matmul(
                ps[:], cbr[:, o * P:(o + 1) * P], ones1p[:, 0:n],
                start=False, stop=True)
            nc.scalar.copy(convT[:, o, c0:c1], ps[:])

    def epi_mix(c0, c1, rb):
        # attention division + routed mix + output DMA for [c0, c1)
        nc.vector.reciprocal(rz[:, :, c0:c1], ZN[:, 0:DT, c0:c1])
        nc.vector.tensor_tensor(out=attnT[:, :, c0:c1],
                                in0=ZN[:, DT:2 * DT, c0:c1],
                                in1=rz[:, :, c0:c1], op=OP.mult)
        nc.scalar.mul(tmp[:, :, c0:c1], mT[:, :, c0:c1], rb[:, 0:1])
        nc.vector.scalar_tensor_tensor(
            out=acc[:, :, c0:c1], in0=attnT[:, :, c0:c1], scalar=rb[:, 1:2],
            in1=tmp[:, :, c0:c1], op0=OP.mult, op1=OP.add)
        nc.vector.scalar_tensor_tensor(
            out=y4[:, :, c0:c1], in0=convT[:, :, c0:c1], scalar=rb[:, 2:3],
            in1=acc[:, :, c0:c1], op0=OP.mult, op1=OP.add)
        nc.sync.dma_start(
            out=dram["y"][:, c0:c1].rearrange("(a p) n -> p a n", p=P),
            in_=y4[:, :, c0:c1])

    # ---------------- main streaming phase ----------------
    # Epilogue halves pipeline across the stream: half 0's m closes after
    # tile 7, so prep/conv/mix for chunks 0-63 run during tiles 8-10.
    HC = NCH // 2
    rb = None
    for j in range(NJ):
        xt = xts[j]

        # EP[:,0:4]=E^T (exp of logits), EP[:,4:8]=P^T (x*E)
        EP = epp.tile([P, 2 * DT, JT], bf16, tag="EP", name="EP")
        for o in range(DT):
            ps = ps_lg.tile([P, JT], f32, tag="lg", name="lg")
            for k in range(DT):
                nc.tensor.matmul(
                    ps[:], aw[k][:, o * P:(o + 1) * P], xt[:, k],
                    start=(k == 0), stop=(k == DT - 1))
            nc.scalar.activation(out=EP[:, o, :], in_=ps[:], func=AF.Exp)
            if j == 0:
                # startup: per-o mult so DVE begins right after the first exp
                nc.vector.tensor_tensor(
                    out=EP[:, DT + o, :], in0=xt[:, o], in1=EP[:, o, :],
                    op=OP.mult)
        if j > 0:
            nc.vector.tensor_tensor(
                out=EP[:, DT:2 * DT, :], in0=xt[:], in1=EP[:, 0:DT, :],
                op=OP.mult)

        # x chunk sums for this tile on PE (own SBUF port)
        gmm(j)

        # E&P segsum64 (DVE): three bf16 pair-add levels (2x mode), then a
        # small f32 reduce over the remaining 8
        ch0 = j * JC
        epv = EP[:].rearrange("p a (n c) -> p a n c", c=C)
        s1 = scratch.tile([P, 2 * DT, JC, C // 2], bf16, tag="s1",
                          name="s1", bufs=2)
        nc.vector.tensor_tensor(out=s1[:], in0=epv[:, :, :, 0:32],
                                in1=epv[:, :, :, 32:64], op=OP.add)
        s2 = scratch.tile([P, 2 * DT, JC, C // 4], bf16, tag="s2",
                          name="s2", bufs=2)
        nc.vector.tensor_tensor(out=s2[:], in0=s1[:, :, :, 0:16],
                                in1=s1[:, :, :, 16:32], op=OP.add)
        s3 = scratch.tile([P, 2 * DT, JC, C // 8], bf16, tag="s3",
                          name="s3", bufs=2)
        nc.vector.tensor_tensor(out=s3[:], in0=s2[:, :, :, 0:8],
                                in1=s2[:, :, :, 8:16], op=OP.add)
        nc.vector.reduce_sum(out=ZN[:, :, ch0:ch0 + JC], in_=s3[:], axis=AX.X)

        if j == 0:
            rb = emit_router()
        elif j == 8:
            m_close(0)
            epi_prep(0, HC)
        elif j == 9:
            epi_conv(0, HC)
        elif j == 10:
            epi_mix(0, HC, rb)

    # second half epilogue after the stream
    m_close(1)
    epi_prep(HC, NCH)
    epi_conv(HC, NCH)
    epi_mix(HC, NCH, rb)


def _build(loop_iters=None):
    import concourse.bass as bass
    from concourse import bacc
    import concourse.mybir as mybir
    import concourse.tile as tile

    f32 = mybir.dt.float32
    bf16 = mybir.dt.bfloat16

    nc = bacc.Bacc(None, target_bir_lowering=False)
    dram = {
        "xT": nc.dram_tensor("xT", [D, S], bf16, kind="ExternalInput"),
        "xn": nc.dram_tensor("xn", [S, D], bf16, kind="ExternalInput"),
        "attn_w": nc.dram_tensor("attn_w", [D, D], bf16, kind="ExternalInput"),
        "w0T": nc.dram_tensor("w0T", [D, D], bf16, kind="ExternalInput"),
        "w1T": nc.dram_tensor("w1T", [D, D], bf16, kind="ExternalInput"),
        "w2T": nc.dram_tensor("w2T", [D, D], bf16, kind="ExternalInput"),
        "fpad": nc.dram_tensor("fpad", [D, NCH + 1], f32, kind="ExternalInput"),
        "lpad": nc.dram_tensor("lpad", [D, NCH + 1], f32, kind="ExternalInput"),
        "router_w1": nc.dram_tensor("router_w1", [D, HID], f32, kind="ExternalInput"),
        "router_b1": nc.dram_tensor("router_b1", [1, HID], f32, kind="ExternalInput"),
        "router_w2": nc.dram_tensor("router_w2", [HID, NEXP], f32, kind="ExternalInput"),
        "router_b2": nc.dram_tensor("router_b2", [1, NEXP], f32, kind="ExternalInput"),
        "conv_b_row": nc.dram_tensor("conv_b_row", [1, D], f32, kind="ExternalInput"),
        "G": nc.dram_tensor("G", [P, 2 * P], bf16, kind="ExternalInput"),
        "ident": nc.dram_tensor("ident", [P, P], bf16, kind="ExternalInput"),
        "y": nc.dram_tensor("y", [D, NCH], f32, kind="ExternalOutput"),
    }
    from contextlib import ExitStack
    with tile.TileContext(nc) as tc:
        with ExitStack() as ctx:
            pools = _make_pools(ctx, tc)
            if loop_iters is None:
                _emit_body(pools, nc, tc, dram, mybir)
            else:
                ET = mybir.EngineType
                with tc.For_i(0, loop_iters, 1,
                              hint_engines=(ET.PE, ET.DVE, ET.Activation,
                                            ET.SP)):
                    _emit_body(pools, nc, tc, dram, mybir)
    nc.finalize()
    return nc


def _host_prep(inputs):
    """Build per-core input maps from full inputs."""
    x = np.asarray(inputs["x"], dtype=np.float32)
    attn_w = np.asarray(inputs["attn_w"], dtype=np.float32)
    conv_w = np.asarray(inputs["conv_w"], dtype=np.float32)
    conv_b = np.asarray(inputs["conv_b"], dtype=np.float32)
    rw1 = np.asarray(inputs["router_w1"], dtype=np.float32)
    rb1 = np.asarray(inputs["router_b1"], dtype=np.float32)
    rw2 = np.asarray(inputs["router_w2"], dtype=np.float32)
    rb2 = np.asarray(inputs["router_b2"], dtype=np.float32)

    aw_bf = np.ascontiguousarray(attn_w).astype(BF16)
    w0T = np.ascontiguousarray(conv_w[:, :, 0].T).astype(BF16)
    w1T = np.ascontiguousarray(conv_w[:, :, 1].T).astype(BF16)
    w2T = np.ascontiguousarray(conv_w[:, :, 2].T).astype(BF16)
    rb1_2d = rb1.reshape(1, HID)
    rb2_2d = rb2.reshape(1, NEXP)
    cb_row = conv_b.reshape(1, D)
    G = np.zeros((P, 2 * P), BF16)
    G[0:C, P] = 1.0
    G[C:P, P + 1] = 1.0
    ident = np.eye(P, dtype=np.float32).astype(BF16)

    in_maps = []
    for b in range(B):
        xb = x[b]
        F = xb[0::C]            # [NCH, D]
        L = xb[C - 1::C]
        fpad = np.zeros((D, NCH + 1), np.float32)
        fpad[:, 0:NCH] = F.T
        lpad = np.zeros((D, NCH + 1), np.float32)
        lpad[:, 1:NCH + 1] = L.T
        in_maps.append({
            "xT": np.ascontiguousarray(xb.T).astype(BF16),
            "xn": xb.astype(BF16),
            "attn_w": aw_bf,
            "w0T": w0T, "w1T": w1T, "w2T": w2T,
            "fpad": fpad, "lpad": lpad,
            "router_w1": rw1, "router_b1": rb1_2d,
            "router_w2": rw2, "router_b2": rb2_2d,
            "conv_b_row": cb_row, "G": G, "ident": ident,
        })
    return in_maps


def kernel(**inputs):
    from concourse.bass_utils import run_bass_kernel_spmd

    if "nc" not in _CACHE:
        _CACHE["nc"] = _build()
    nc = _CACHE["nc"]
    in_maps = _host_prep(inputs)
    res = run_bass_kernel_spmd(nc, in_maps, list(range(N_CORES)))
    out = np.stack([np.ascontiguousarray(res.results[b]["y"].T)
                    for b in range(B)])
    return out.astype(np.float32)


if __name__ == "__main__":
    rng = np.random.default_rng(0)
    fake = {
        "x": rng.standard_normal((B, S, D), dtype=np.float32),
        "attn_w": rng.standard_normal((D, D), dtype=np.float32) / np.sqrt(D),
        "attn_b": np.zeros(D, np.float32),
        "conv_w": rng.standard_normal((D, D, 3), dtype=np.float32) / np.sqrt(3 * D),
        "conv_b": np.zeros(D, np.float32),
        "router_w1": rng.standard_normal((D, HID), dtype=np.float32) / np.sqrt(D),
        "router_b1": np.zeros(HID, np.float32),
        "router_w2": rng.standard_normal((HID, NEXP), dtype=np.float32) / np.sqrt(HID),
        "router_b2": np.zeros(NEXP, np.float32),
    }
    y = kernel(**fake)
    print("kernel out", y.shape, y.dtype, np.abs(y).max())


# revision 18
# speedup vs baseline: 1.2376x; 1.0330x over previous
"""Trainium2 Bass kernel for nn_EnterpriseNeuralMemory (scatter_memory).

Sharding: data-parallel over batch — 8 batch elements, one per NeuronCore.
No collectives needed (router mean is per-batch-element and chunk pooling is
chunk-local).

Per-core algorithm (batch element b, transposed layouts = [feature, pos]):
  logitsT = attn_w.T @ x.T          (PE, bf16, 16 pos-tiles of 512)
  E^T = exp(logitsT)                (ACT, PSUM->SBUF bf16)
  P^T = x^T * E^T                   (DVE bf16 2x)
  Z,N,M = segsum64(E^T, P^T, x^T)   (DVE: 3 bf16 pair-add levels + f32 red,
                                     two stream tiles batched per op to
                                     amortize per-instruction overhead)
  conv_pool  = W0@(m+u/64) + W1@m + W2@(m+v/64) + conv_b
               (boundary algebra: u/v from strided firsts/lasts columns)
  router: mean of chunk-first tokens -> 2-layer MLP -> softmax(3)
  out = r0*m + r1*(N/Z) + r2*conv_pool     with m = M/64

Engine notes (HW-measured): DVE and Pool(GpSimd) share SBUF ports, so Pool
offloading buys nothing — all elementwise work rides one port budget; PE
matmuls cost ~266ns per 512-col bf16 matmul incl the stationary reload, so
moving chunk-sums to PE loses too. x is streamed once (transposed bf16).
"""

import numpy as np
import ml_dtypes

BF16 = ml_dtypes.bfloat16

B, S, D = 8, 8192, 512
C = 64                      # chunk size
NCH = S // C                # 128 chunks
P = 128                     # partitions
DT = D // P                 # 4 feature tiles
JT = 512                    # positions per matmul tile
NJ = S // JT                # 16 pos-tiles
NPAIR = NJ // 2             # 8 stream pairs (2 tiles per DVE batch)
PC = 2 * JT // C            # 16 chunks per pair
HID, NEXP = 128, 3

N_CORES = 8

_CACHE = {}


def _make_pools(ctx, tc):
    return {
        "consts": ctx.enter_context(tc.tile_pool(name="consts", bufs=1)),
        "xtp": ctx.enter_context(tc.tile_pool(name="xtp", bufs=NPAIR)),
        "epp": ctx.enter_context(tc.tile_pool(name="epp", bufs=2)),
        "grids": ctx.enter_context(tc.tile_pool(name="grids", bufs=1)),
        "scratch": ctx.enter_context(tc.tile_pool(name="scratch", bufs=1)),
        "ps_lg": ctx.enter_context(tc.tile_pool(name="ps_lg", bufs=6, space="PSUM")),
        "ps_epi": ctx.enter_context(tc.tile_pool(name="ps_epi", bufs=2, space="PSUM")),
    }


def _emit_body(pools, nc, tc, dram, mybir):
    """Emit one full forward pass for one core."""
    f32 = mybir.dt.float32
    bf16 = mybir.dt.bfloat16
    AF = mybir.ActivationFunctionType
    OP = mybir.AluOpType
    AX = mybir.AxisListType

    consts = pools["consts"]
    xtp = pools["xtp"]
    epp = pools["epp"]
    grids = pools["grids"]
    scratch = pools["scratch"]
    ps_lg = pools["ps_lg"]
    ps_epi = pools["ps_epi"]

    # [512, X] dram tensors load as one [128, 4, X] tile each (one DMA).
    def load4(src, cols, dtype, nm):
        t = consts.tile([P, DT, cols], dtype, tag=nm, name=nm)
        nc.sync.dma_start(
            out=t[:], in_=src[:, :].rearrange("(a p) c -> p a c", p=P))
        return t

    # stream pairs: tile [P, DT, 2*JT], two DMAs per pair (halves)
    def xt_dma(p, half):
        nc.sync.dma_start(
            out=xt2s[p][:, :, half * JT:(half + 1) * JT],
            in_=dram["xT"][:, (2 * p + half) * JT:(2 * p + half + 1) * JT]
                .rearrange("(a p) c -> p a c", p=P))

    xt2s = [xtp.tile([P, DT, 2 * JT], bf16, tag="xt", name=f"xt{p}")
            for p in range(NPAIR)]
    xt_dma(0, 0)

    aw = []
    for k in range(DT):
        t = consts.tile([P, D], bf16, tag=f"aw{k}", name=f"aw{k}")
        nc.sync.dma_start(out=t[:], in_=dram["attn_w"][k * P:(k + 1) * P, :])
        aw.append(t)
    xt_dma(0, 1)

    # router / boundary inputs (small; needed within the first few tiles)
    fp4 = load4(dram["fpad"], NCH + 1, f32, "fp4")
    lp4 = load4(dram["lpad"], NCH + 1, f32, "lp4")
    rw14 = load4(dram["router_w1"], HID, f32, "rw14")
    rw1 = [rw14[:, k] for k in range(DT)]
    rb1 = consts.tile([1, HID], f32, tag="rb1", name="rb1")
    nc.sync.dma_start(out=rb1[:], in_=dram["router_b1"][:])
    rw2 = consts.tile([HID, NEXP], f32, tag="rw2", name="rw2")
    nc.sync.dma_start(out=rw2[:], in_=dram["router_w2"][:])
    rb2 = consts.tile([1, NEXP], f32, tag="rb2", name="rb2")
    nc.sync.dma_start(out=rb2[:], in_=dram["router_b2"][:])
    ones11 = consts.tile([1, 1], f32, tag="ones11", name="ones11")
    nc.vector.memset(ones11[:], 1.0)
    ones1p = consts.tile([1, P], f32, tag="ones1p", name="ones1p")
    nc.vector.memset(ones1p[:], 1.0)

    # remaining stream DMAs
    for p in range(1, NPAIR):
        xt_dma(p, 0)
        xt_dma(p, 1)

    # conv weights (needed from the first epilogue quarter onward)
    wT = {}
    for w in range(3):
        w4 = load4(dram[f"w{w}T"], D, bf16, f"w{w}T4")
        wT[w] = [w4[:, k] for k in range(DT)]
    cbr = consts.tile([1, D], f32, tag="cbr", name="cbr")
    nc.sync.dma_start(out=cbr[:], in_=dram["conv_b_row"][:])

    # conv boundary terms: u_i = L_{i-1}-L_i, v_i = F_{i+1}-F_i
    u = grids.tile([P, DT, NCH], f32, tag="u", name="u")
    nc.vector.tensor_tensor(out=u[:], in0=lp4[:, :, 0:NCH],
                            in1=lp4[:, :, 1:NCH + 1], op=OP.subtract)
    v = grids.tile([P, DT, NCH], f32, tag="v", name="v")
    nc.vector.tensor_tensor(out=v[:], in0=fp4[:, :, 1:NCH + 1],
                            in1=fp4[:, :, 0:NCH], op=OP.subtract)

    def emit_router():
        # router MLP + softmax + broadcast of r; emitted after the first
        # matmuls so its PE ops never block the stream start
        xfs = grids.tile([P, DT], f32, tag="xfs", name="xfs")
        nc.vector.reduce_sum(out=xfs[:], in_=fp4[:, :, 0:NCH], axis=AX.X)
        xf = grids.tile([P, DT], f32, tag="xf", name="xf")
        nc.scalar.mul(xf[:], xfs[:], 1.0 / NCH)
        ps_h = ps_epi.tile([P, 1], f32, tag="epi", name="epi")
        for k in range(DT):
            nc.tensor.matmul(ps_h[:], rw1[k][:], xf[:, k:k + 1],
                             start=(k == 0), stop=False)
        nc.tensor.matmul(ps_h[:], rb1[:], ones11[:], start=False, stop=True)
        hsb = grids.tile([P, 1], f32, tag="hsb", name="hsb")
        nc.scalar.activation(out=hsb[:], in_=ps_h[:], func=AF.Relu)
        ps_r = ps_epi.tile([1, NEXP], f32, tag="epi", name="epi")
        nc.tensor.matmul(ps_r[:], hsb[:], rw2[:], start=True, stop=False)
        nc.tensor.matmul(ps_r[:], ones11[:], rb2[:], start=False, stop=True)
        rmax = grids.tile([1, 1], f32, tag="rmax", name="rmax")
        nc.vector.reduce_max(out=rmax[:], in_=ps_r[:], axis=AX.X)
        nrmax = grids.tile([1, 1], f32, tag="nrmax", name="nrmax")
        nc.vector.tensor_scalar_mul(nrmax[:], rmax[:], -1.0)
        er = grids.tile([1, NEXP], f32, tag="er", name="er")
        nc.scalar.activation(out=er[:], in_=ps_r[:], func=AF.Exp,
                             bias=nrmax[:])
        rsum = grids.tile([1, 1], f32, tag="rsum", name="rsum")
        nc.vector.reduce_sum(out=rsum[:], in_=er[:], axis=AX.X)
        rrec = grids.tile([1, 1], f32, tag="rrec", name="rrec")
        nc.vector.reciprocal(rrec[:], rsum[:])
        rvec = grids.tile([1, NEXP], f32, tag="rvec", name="rvec")
        nc.vector.tensor_scalar_mul(rvec[:], er[:], rrec[:])
        ps_b = ps_epi.tile([P, NEXP], f32, tag="epi", name="epi")
        nc.tensor.matmul(ps_b[:], ones1p[:], rvec[:], start=True, stop=True)
        rb = grids.tile([P, NEXP], f32, tag="rb", name="rb")
        nc.scalar.copy(rb[:], ps_b[:])
        return rb

    # segsum result grids: ZN[:,0:4]=Z (softmax denom), ZN[:,4:8]=N (numer)
    ZN = grids.tile([P, 2 * DT, NCH], f32, tag="ZN", name="ZN")
    Mc = grids.tile([P, DT, NCH], f32, tag="Mc", name="Mc")

    # epilogue grids (written in chunk-range quarters)
    mT = grids.tile([P, DT, NCH], f32, tag="mT", name="mT")
    mTb = grids.tile([P, DT, NCH], bf16, tag="mTb", name="mTb")
    aTb = grids.tile([P, DT, NCH], bf16, tag="aTb", name="aTb")
    cTb = grids.tile([P, DT, NCH], bf16, tag="cTb", name="cTb")
    convT = grids.tile([P, DT, NCH], f32, tag="convT", name="convT")
    rz = grids.tile([P, DT, NCH], f32, tag="rz", name="rz")
    attnT = grids.tile([P, DT, NCH], f32, tag="attnT", name="attnT")
    tmp = grids.tile([P, DT, NCH], f32, tag="tmp", name="tmp")
    acc = grids.tile([P, DT, NCH], f32, tag="acc", name="acc")
    y4 = grids.tile([P, DT, NCH], f32, tag="y4", name="y4")

    def epi_prep(c0, c1):
        # conv-expert inputs for chunk range [c0, c1): m, m+u/64, m+v/64
        nc.scalar.mul(mT[:, :, c0:c1], Mc[:, :, c0:c1], 1.0 / C)
        nc.scalar.copy(mTb[:, :, c0:c1], mT[:, :, c0:c1])
        nc.vector.scalar_tensor_tensor(
            out=aTb[:, :, c0:c1], in0=u[:, :, c0:c1], scalar=1.0 / C,
            in1=mT[:, :, c0:c1], op0=OP.mult, op1=OP.add)
        nc.vector.scalar_tensor_tensor(
            out=cTb[:, :, c0:c1], in0=v[:, :, c0:c1], scalar=1.0 / C,
            in1=mT[:, :, c0:c1], op0=OP.mult, op1=OP.add)

    def epi_conv(c0, c1):
        # conv expert matmuls for chunk range [c0, c1)
        n = c1 - c0
        for o in range(DT):
            ps = ps_epi.tile([P, n], f32, tag="epi", name="epi")
            first = True
            for w, rhs4 in ((0, aTb), (1, mTb), (2, cTb)):
                for k in range(DT):
                    nc.tensor.matmul(
                        ps[:], wT[w][k][:, o * P:(o + 1) * P],
                        rhs4[:, k, c0:c1], start=first, stop=False)
                    first = False
            nc.tensor.matmul(
                ps[:], cbr[:, o * P:(o + 1) * P], ones1p[:, 0:n],
                start=False, stop=True)
            nc.scalar.copy(convT[:, o, c0:c1], ps[:])

    def epi_mix(c0, c1, rb):
        # attention division + routed mix + output DMA for [c0, c1)
        nc.vector.reciprocal(rz[:, :, c0:c1], ZN[:, 0:DT, c0:c1])
        nc.vector.tensor_tensor(out=attnT[:, :, c0:c1],
                                in0=ZN[:, DT:2 * DT, c0:c1],
                                in1=rz[:, :, c0:c1], op=OP.mult)
        nc.scalar.mul(tmp[:, :, c0:c1], mT[:, :, c0:c1], rb[:, 0:1])
        nc.vector.scalar_tensor_tensor(
            out=acc[:, :, c0:c1], in0=attnT[:, :, c0:c1], scalar=rb[:, 1:2],
            in1=tmp[:, :, c0:c1], op0=OP.mult, op1=OP.add)
        nc.vector.scalar_tensor_tensor(
            out=y4[:, :, c0:c1], in0=convT[:, :, c0:c1], scalar=rb[:, 2:3],
            in1=acc[:, :, c0:c1], op0=OP.mult, op1=OP.add)
        nc.sync.dma_start(
            out=dram["y"][:, c0:c1].rearrange("(a p) n -> p a n", p=P),
            in_=y4[:, :, c0:c1])

    # ---------------- main streaming phase (two tiles per pair) ----------
    mcred_pending = []
    rb = None
    for p in range(NPAIR):
        xt2 = xt2s[p]

        # EP[:,0:4]=E^T (exp of logits), EP[:,4:8]=P^T (x*E); both halves
        EP = epp.tile([P, 2 * DT, 2 * JT], bf16, tag="EP", name="EP")
        for half in range(2):
            for o in range(DT):
                ps = ps_lg.tile([P, JT], f32, tag="lg", name="lg")
                for k in range(DT):
                    nc.tensor.matmul(
                        ps[:], aw[k][:, o * P:(o + 1) * P],
                        xt2[:, k, half * JT:(half + 1) * JT],
                        start=(k == 0), stop=(k == DT - 1))
                nc.scalar.activation(
                    out=EP[:, o, half * JT:(half + 1) * JT], in_=ps[:],
                    func=AF.Exp)
            if p == 0:
                # startup: per-half mult so DVE begins after 4 exps, not 8
                nc.vector.tensor_tensor(
                    out=EP[:, DT:2 * DT, half * JT:(half + 1) * JT],
                    in0=xt2[:, :, half * JT:(half + 1) * JT],
                    in1=EP[:, 0:DT, half * JT:(half + 1) * JT], op=OP.mult)
        if p > 0:
            nc.vector.tensor_tensor(
                out=EP[:, DT:2 * DT, :], in0=xt2[:], in1=EP[:, 0:DT, :],
                op=OP.mult)

        # E&P segsum64 (DVE): three bf16 pair-add levels, small f32 reduce
        ch0 = p * PC
        epv = EP[:].rearrange("p a (n c) -> p a n c", c=C)
        s1 = scratch.tile([P, 2 * DT, PC, C // 2], bf16, tag="s1",
                          name="s1", bufs=2)
        nc.vector.tensor_tensor(out=s1[:], in0=epv[:, :, :, 0:32],
                                in1=epv[:, :, :, 32:64], op=OP.add)
        s2 = scratch.tile([P, 2 * DT, PC, C // 4], bf16, tag="s2",
                          name="s2", bufs=2)
        nc.vector.tensor_tensor(out=s2[:], in0=s1[:, :, :, 0:16],
                                in1=s1[:, :, :, 16:32], op=OP.add)
        s3 = scratch.tile([P, 2 * DT, PC, C // 8], bf16, tag="s3",
                          name="s3", bufs=2)
        nc.vector.tensor_tensor(out=s3[:], in0=s2[:, :, :, 0:8],
                                in1=s2[:, :, :, 8:16], op=OP.add)
        nc.vector.reduce_sum(out=ZN[:, :, ch0:ch0 + PC], in_=s3[:], axis=AX.X)

        # x segsum64 (DVE, batched pair)
        xv = xt2[:].rearrange("p a (n c) -> p a n c", c=C)
        t1 = scratch.tile([P, DT, PC, C // 2], bf16, tag="t1",
                          name="t1", bufs=2)
        nc.vector.tensor_tensor(out=t1[:], in0=xv[:, :, :, 0:32],
                                in1=xv[:, :, :, 32:64], op=OP.add)
        t2 = scratch.tile([P, DT, PC, C // 4], bf16, tag="t2",
                          name="t2", bufs=2)
        nc.vector.tensor_tensor(out=t2[:], in0=t1[:, :, :, 0:16],
                                in1=t1[:, :, :, 16:32], op=OP.add)
        t3 = scratch.tile([P, DT, PC, C // 8], bf16, tag="t3",
                          name="t3", bufs=2)
        nc.vector.tensor_tensor(out=t3[:], in0=t2[:, :, :, 0:8],
                                in1=t2[:, :, :, 8:16], op=OP.add)
        # defer the Mc reduce by one pair (epilogue never blocks the tree)
        while mcred_pending:
            pt3, pch0 = mcred_pending.pop(0)
            nc.vector.reduce_sum(out=Mc[:, :, pch0:pch0 + PC], in_=pt3[:],
                                 axis=AX.X)
        if p == NPAIR - 1:
            nc.vector.reduce_sum(out=Mc[:, :, ch0:ch0 + PC], in_=t3[:],
                                 axis=AX.X)
        else:
            mcred_pending.append((t3, ch0))

        QC = NCH // 4
        if p == 0:
            rb = emit_router()
        elif p in (3, 5, 7):
            q = (p - 3) // 2
            epi_prep(q * QC, (q + 1) * QC)
            epi_conv(q * QC, (q + 1) * QC)
        elif p in (4, 6):
            q = (p - 4) // 2
            epi_mix(q * QC, (q + 1) * QC, rb)

    # tail: mix q2, full q3
    QC = NCH // 4
    epi_mix(2 * QC, 3 * QC, rb)
    epi_prep(3 * QC, NCH)
    epi_conv(3 * QC, NCH)
    epi_mix(3 * QC, NCH, rb)


def _build(loop_iters=None):
    import concourse.bass as bass
    from concourse import bacc
    import concourse.mybir as mybir
    import concourse.tile as tile

    f32 = mybir.dt.float32
    bf16 = mybir.dt.bfloat16

    nc = bacc.Bacc(None, target_bir_lowering=False)
    dram = {
        "xT": nc.dram_tensor("xT", [D, S], bf16, kind="ExternalInput"),
        "attn_w": nc.dram_tensor("attn_w", [D, D], bf16, kind="ExternalInput"),
        "w0T": nc.dram_tensor("w0T", [D, D], bf16, kind="ExternalInput"),
        "w1T": nc.dram_tensor("w1T", [D, D], bf16, kind="ExternalInput"),
        "w2T": nc.dram_tensor("w2T", [D, D], bf16, kind="ExternalInput"),
        "fpad": nc.dram_tensor("fpad", [D, NCH + 1], f32, kind="ExternalInput"),
        "lpad": nc.dram_tensor("lpad", [D, NCH + 1], f32, kind="ExternalInput"),
        "router_w1": nc.dram_tensor("router_w1", [D, HID], f32, kind="ExternalInput"),
        "router_b1": nc.dram_tensor("router_b1", [1, HID], f32, kind="ExternalInput"),
        "router_w2": nc.dram_tensor("router_w2", [HID, NEXP], f32, kind="ExternalInput"),
        "router_b2": nc.dram_tensor("router_b2", [1, NEXP], f32, kind="ExternalInput"),
        "conv_b_row": nc.dram_tensor("conv_b_row", [1, D], f32, kind="ExternalInput"),
        "y": nc.dram_tensor("y", [D, NCH], f32, kind="ExternalOutput"),
    }
    from contextlib import ExitStack
    with tile.TileContext(nc) as tc:
        with ExitStack() as ctx:
            pools = _make_pools(ctx, tc)
            if loop_iters is None:
                _emit_body(pools, nc, tc, dram, mybir)
            else:
                ET = mybir.EngineType
                with tc.For_i(0, loop_iters, 1,
                              hint_engines=(ET.PE, ET.DVE, ET.Activation,
                                            ET.SP)):
                    _emit_body(pools, nc, tc, dram, mybir)
    nc.finalize()
    return nc


def _host_prep(inputs):
    """Build per-core input maps from full inputs."""
    x = np.asarray(inputs["x"], dtype=np.float32)
    attn_w = np.asarray(inputs["attn_w"], dtype=np.float32)
    conv_w = np.asarray(inputs["conv_w"], dtype=np.float32)
    conv_b = np.asarray(inputs["conv_b"], dtype=np.float32)
    rw1 = np.asarray(inputs["router_w1"], dtype=np.float32)
    rb1 = np.asarray(inputs["router_b1"], dtype=np.float32)
    rw2 = np.asarray(inputs["router_w2"], dtype=np.float32)
    rb2 = np.asarray(inputs["router_b2"], dtype=np.float32)

    aw_bf = np.ascontiguousarray(attn_w).astype(BF16)
    w0T = np.ascontiguousarray(conv_w[:, :, 0].T).astype(BF16)
    w1T = np.ascontiguousarray(conv_w[:, :, 1].T).astype(BF16)
    w2T = np.ascontiguousarray(conv_w[:, :, 2].T).astype(BF16)
    rb1_2d = rb1.reshape(1, HID)
    rb2_2d = rb2.reshape(1, NEXP)
    cb_row = conv_b.reshape(1, D)

    in_maps = []
    for b in range(B):
        xb = x[b]
        F = xb[0::C]            # [NCH, D]
        L = xb[C - 1::C]
        fpad = np.zeros((D, NCH + 1), np.float32)
        fpad[:, 0:NCH] = F.T
        lpad = np.zeros((D, NCH + 1), np.float32)
        lpad[:, 1:NCH + 1] = L.T
        in_maps.append({
            "xT": np.ascontiguousarray(xb.T).astype(BF16),
            "attn_w": aw_bf,
            "w0T": w0T, "w1T": w1T, "w2T": w2T,
            "fpad": fpad, "lpad": lpad,
            "router_w1": rw1, "router_b1": rb1_2d,
            "router_w2": rw2, "router_b2": rb2_2d,
            "conv_b_row": cb_row,
        })
    return in_maps


def kernel(**inputs):
    from concourse.bass_utils import run_bass_kernel_spmd

    if "nc" not in _CACHE:
        _CACHE["nc"] = _build()
    nc = _CACHE["nc"]
    in_maps = _host_prep(inputs)
    res = run_bass_kernel_spmd(nc, in_maps, list(range(N_CORES)))
    out = np.stack([np.ascontiguousarray(res.results[b]["y"].T)
                    for b in range(B)])
    return out.astype(np.float32)


if __name__ == "__main__":
    rng = np.random.default_rng(0)
    fake = {
        "x": rng.standard_normal((B, S, D), dtype=np.float32),
        "attn_w": rng.standard_normal((D, D), dtype=np.float32) / np.sqrt(D),
        "attn_b": np.zeros(D, np.float32),
        "conv_w": rng.standard_normal((D, D, 3), dtype=np.float32) / np.sqrt(3 * D),
        "conv_b": np.zeros(D, np.float32),
        "router_w1": rng.standard_normal((D, HID), dtype=np.float32) / np.sqrt(D),
        "router_b1": np.zeros(HID, np.float32),
        "router_w2": rng.standard_normal((HID, NEXP), dtype=np.float32) / np.sqrt(HID),
        "router_b2": np.zeros(NEXP, np.float32),
    }
    y = kernel(**fake)
    print("kernel out", y.shape, y.dtype, np.abs(y).max())
